# revision 29
# baseline (speedup 1.0000x reference)
"""Trainium2 Bass kernel for nn_Mlp_84275848282705 (SmoothQuant-style quantized ViT MLP).

v2: data-parallel over tokens (12608 = 8 x 1576). Changes vs v1 baseline:
- fc2 output computed in transposed [C, tokens] layout: the epilogue
  (s_h*s2[c] scale + b2[c] bias) becomes per-partition Scalar-engine work
  straight out of PSUM, and the output stays in SBUF until the final quant
  (host transposes back). No out DRAM round-trip.
- h (pre-quant gelu output, must stay f32) is kept SBUF-resident for
  NRES of 24 h-tiles; only the rest spill to DRAM. hq (integers, bf16-exact)
  is produced just-in-time per fc2 chunk into a small ring.
- w1 is loaded once (f32-resident through colmax/s1/quant).
- Big-reciprocal fix: 1/s1 computed on a [128,24] column form (DRAM-bounce)
  instead of a 25us DVE iterative divide on [128,3072].
- Symmetric-quant clamps dropped (|round(w/s)| <= 127 by construction).
"""
import sys

sys.path.insert(0, "/opt/trn_rl_repo")

import numpy as np

B, N, C, H = 64, 197, 768, 3072
TOK = B * N             # 12608
N_CORES = 8
TLOC = TOK // N_CORES   # 1576
NRES = 12               # h tiles resident in SBUF (of 24); rest spill to DRAM
RND = 12582912.0        # 1.5*2^23: RNE integer-round magic const (valid for |x| <= 2^22)
EPS = 1e-8
INV_LN2 = 1.4426950408889634
LN2 = 0.6931471805599453
R127 = float(np.float32(1.0) / np.float32(127.0))


def _chunks(t_pad, step):
    out, off = [], 0
    while off < t_pad:
        w = min(step, t_pad - off)
        out.append((off, w))
        off += w
    return out


def build(n_cores=N_CORES, t_loc=TLOC):
    import concourse.bacc as bacc
    import concourse.tile as tile
    from concourse import mybir

    F32 = mybir.dt.float32
    t_pad = ((t_loc + 127) // 128) * 128

    nc = bacc.Bacc("TRN2", target_bir_lowering=False, debug=False,
                   enable_asserts=False, num_devices=n_cores)

    io = dict(
        xT=nc.dram_tensor("xT", [C, t_pad], F32, kind="ExternalInput").ap(),
        w1T=nc.dram_tensor("w1T", [C, H], F32, kind="ExternalInput").ap(),
        w2T=nc.dram_tensor("w2T", [H, C], F32, kind="ExternalInput").ap(),
        b1=nc.dram_tensor("b1", [H], F32, kind="ExternalInput").ap(),
        b2=nc.dram_tensor("b2", [C], F32, kind="ExternalInput").ap(),
        # transposed output: [C, t_pad]; host transposes back
        out_e=nc.dram_tensor("out", [C, t_pad], F32, kind="ExternalOutput").ap(),
    )

    with tile.TileContext(nc) as tc:
        _emit(nc, tc, io, n_cores, t_loc, t_pad)
    nc.compile()
    return nc


def _emit(nc, tc, io, n_cores, t_loc, t_pad):
    from contextlib import ExitStack
    from concourse import mybir, bass_isa
    from concourse.tile import add_dep_helper

    F32 = mybir.dt.float32
    BF16 = mybir.dt.bfloat16
    AT = mybir.AluOpType
    AFT = mybir.ActivationFunctionType
    AX = mybir.AxisListType.X
    ROP = bass_isa.ReduceOp
    RG = [list(range(n_cores))]

    xT, w1T, w2T, b1, b2, out_e = (io[k] for k in
                                   ("xT", "w1T", "w2T", "b1", "b2", "out_e"))

    CH = _chunks(t_pad, 512)
    n_spill = 24 - NRES

    DVE, ACT, GPS, SYNC = nc.vector, nc.scalar, nc.gpsimd, nc.sync
    MM = nc.tensor.matmul

    with ExitStack() as ctx:
        const = ctx.enter_context(tc.tile_pool(name="const", bufs=1))
        dram = ctx.enter_context(tc.tile_pool(name="dram", bufs=1, space="DRAM"))
        psp = ctx.enter_context(tc.tile_pool(name="ps", bufs=8, space="PSUM"))

        # DRAM scratch (collectives + layout bounces + h spill)
        st_in = dram.tile([1, 2 * C], F32)
        st_out = dram.tile([1, 2 * C], F32)
        sc_in = dram.tile([1, 8], F32)
        sc_out = dram.tile([1, 8], F32)
        sc_in2 = dram.tile([1, 8], F32)
        sc_out2 = dram.tile([1, 8], F32)
        s1row = dram.tile([1, H], F32)
        s2row = dram.tile([1, C], F32)
        i2row = dram.tile([1, C], F32)
        hd = dram.tile([n_spill, 128, t_pad], F32)

        # small const tiles
        b1t = const.tile([128, 24], F32)
        SYNC.dma_start(out=b1t[:], in_=b1.rearrange("(k p) -> p k", p=128))
        b2t = const.tile([128, 6], F32)
        SYNC.dma_start(out=b2t[:], in_=b2.rearrange("(k p) -> p k", p=128))

        stat_max = const.tile([128, 6], F32)
        stat_nm = const.tile([128, 6], F32)
        stat_abs = const.tile([128, 6], F32)
        wcol = const.tile([128, 6], F32)
        n_hcols = NRES + 4 * n_spill
        habs_cols = const.tile([128, n_hcols], F32)
        omax_cols = const.tile([128, 24], F32)
        onm_cols = const.tile([128, 24], F32)
        invs2_bc = const.tile([128, C], F32)
        s2col = const.tile([128, 6], F32)
        i2col = const.tile([128, 6], F32)
        es2 = const.tile([128, 6], F32)
        s1col = const.tile([128, 24], F32)
        A1 = const.tile([128, 24], F32)

        # ---- small-tile math helpers (DVE has no divide: reciprocal+Newton) ----
        _mtmp = [0]

        def _tmp(shape):
            t = const.tile(list(shape), F32, name=f"mt{_mtmp[0]}")
            _mtmp[0] += 1
            return t

        def recip_newton(out, b):
            """out = 1/b to ~0.5 ulp (InstReciprocal + one Newton step)."""
            DVE.reciprocal(out=out[:], in_=b[:])
            t = _tmp(b.shape)
            DVE.tensor_tensor(out=t[:], in0=b[:], in1=out[:], op=AT.mult)
            DVE.tensor_scalar(out=t[:], in0=t[:], scalar1=-1.0, scalar2=2.0,
                              op0=AT.mult, op1=AT.add)
            DVE.tensor_tensor(out=out[:], in0=out[:], in1=t[:], op=AT.mult)

        def div_const(out, a, c, eps_clamp=False):
            """out = a / c (python const), correctly rounded via Newton residual."""
            r = float(np.float32(1.0) / np.float32(c))
            q0 = _tmp(a.shape)
            DVE.tensor_scalar(out=q0[:], in0=a[:], scalar1=r, scalar2=None,
                              op0=AT.mult)
            e = _tmp(a.shape)
            DVE.scalar_tensor_tensor(out=e[:], in0=q0[:], scalar=-float(c), in1=a[:],
                                     op0=AT.mult, op1=AT.add)
            DVE.scalar_tensor_tensor(out=out[:], in0=e[:], scalar=r, in1=q0[:],
                                     op0=AT.mult, op1=AT.add)
            if eps_clamp:
                DVE.tensor_scalar(out=out[:], in0=out[:], scalar1=EPS, scalar2=None,
                                  op0=AT.max)

        # ============ Phase A: x stats -> AR1 ====================================
        xload_insts = []
        with tc.tile_pool(name="xs0", bufs=2) as xs0:
            for ct in range(6):
                xt = xs0.tile([128, t_pad], F32, tag="x0")
                xload_insts.append(
                    SYNC.dma_start(out=xt[:], in_=xT[ct * 128:(ct + 1) * 128, :]))
                DVE.tensor_reduce(out=stat_max[:, ct:ct + 1], in_=xt[:], axis=AX,
                                  op=AT.max)
                DVE.tensor_reduce(out=stat_nm[:, ct:ct + 1], in_=xt[:], axis=AX,
                                  op=AT.min, negate=True)
            # pack/unpack on the ACT engine's HW DMA queue: the Sync queue
            # carries the bulk weight loads and head-of-line blocks tiny DMAs
            pack_insts = []
            pack_insts.append(ACT.dma_start(
                out=st_in[0:1, 0:C].rearrange("a (k p) -> (a p) k", p=128),
                in_=stat_max[:]))
            pack_insts.append(ACT.dma_start(
                out=st_in[0:1, C:2 * C].rearrange("a (k p) -> (a p) k", p=128),
                in_=stat_nm[:]))
            GPS.collective_compute("AllReduce", AT.max, replica_groups=RG,
                                   ins=[st_in.opt()], outs=[st_out.opt()])
            ACT.dma_start(out=stat_max[:],
                          in_=st_out[0:1, 0:C].rearrange("a (k p) -> (a p) k", p=128))
            ACT.dma_start(out=stat_nm[:],
                          in_=st_out[0:1, C:2 * C].rearrange("a (k p) -> (a p) k", p=128))
            DVE.tensor_tensor(out=stat_abs[:], in0=stat_max[:], in1=stat_nm[:],
                              op=AT.max)

        # ============ Phase B: w1 resident; cs chain; s1; w1 quant; x quant ======
        gemm1 = ExitStack()
        xqp = gemm1.enter_context(tc.tile_pool(name="xqp", bufs=1))
        w1qp = gemm1.enter_context(tc.tile_pool(name="w1qp", bufs=1))
        xq = [xqp.tile([128, t_pad], BF16, name=f"xq{i}") for i in range(6)]
        w1q = [w1qp.tile([128, H], BF16, name=f"w1q{i}") for i in range(6)]
        w1load_insts = []
        with tc.tile_pool(name="w1fp", bufs=1) as w1fp, \
             tc.tile_pool(name="w1sc", bufs=2) as w1sc, \
             tc.tile_pool(name="xqs", bufs=3) as xqs, \
             tc.tile_pool(name="s1scr", bufs=1) as s1scr:
            w1f = [w1fp.tile([128, H], F32, name=f"w1f{i}") for i in range(6)]
            s1acc = s1scr.tile([128, H], F32)
            s1b = s1scr.tile([128, H], F32)
            invs1_bc = s1scr.tile([128, H], F32)

            for ct in range(6):
                wl = SYNC.dma_start(out=w1f[ct][:],
                                    in_=w1T[ct * 128:(ct + 1) * 128, :])
                w1load_insts.append(wl)
                if ct == 0:
                    for xl in xload_insts:
                        add_dep_helper(wl.ins, xl.ins,
                                       reason="x stats DMA priority")
                wr = DVE.tensor_reduce(out=wcol[:, ct:ct + 1], in_=w1f[ct][:],
                                       axis=AX, op=AT.max,
                                       apply_absolute_value=True)
                for pk in pack_insts:
                    add_dep_helper(wr.ins, pk.ins,
                                   reason="AR1 pack before w1 colmax on DVE")

            # ---- channel scale cs = pow2-snap(sqrt(gmax/wmax)) ----
            ratio = const.tile([128, 6], F32)
            rw = const.tile([128, 6], F32)
            DVE.reciprocal(out=rw[:], in_=wcol[:])
            DVE.tensor_tensor(out=ratio[:], in0=stat_abs[:], in1=rw[:], op=AT.mult)
            cs_a = const.tile([128, 6], F32)
            ACT.activation(out=cs_a[:], in_=ratio[:], func=AFT.Sqrt)
            rc = const.tile([128, 6], F32)
            DVE.reciprocal(out=rc[:], in_=cs_a[:])
            newt = const.tile([128, 6], F32)
            DVE.tensor_tensor(out=newt[:], in0=ratio[:], in1=rc[:], op=AT.mult)
            DVE.tensor_tensor(out=cs_a[:], in0=cs_a[:], in1=newt[:], op=AT.add)
            DVE.tensor_scalar(out=cs_a[:], in0=cs_a[:], scalar1=0.5, scalar2=None,
                              op0=AT.mult)
            # y = floor(log2(cs)) = round(ln(cs)*(1/ln2) - 0.5)  (RNE round-trick)
            yf = const.tile([128, 6], F32)
            ACT.activation(out=yf[:], in_=cs_a[:], func=AFT.Ln)
            DVE.tensor_scalar(out=yf[:], in0=yf[:], scalar1=INV_LN2,
                              scalar2=0.5, op0=AT.mult, op1=AT.subtract)
            DVE.tensor_scalar(out=yf[:], in0=yf[:], scalar1=RND, scalar2=RND,
                              op0=AT.add, op1=AT.subtract)
            # p = exact 2^y: exp(y*ln2), snapped to exact value at 2^12 scale
            p2 = const.tile([128, 6], F32)
            ACT.activation(out=p2[:], in_=yf[:], func=AFT.Exp, scale=LN2)
            DVE.tensor_scalar(out=p2[:], in0=p2[:], scalar1=4096.0, scalar2=RND,
                              op0=AT.mult, op1=AT.add)
            DVE.tensor_scalar(out=p2[:], in0=p2[:], scalar1=RND,
                              scalar2=1.0 / 4096.0, op0=AT.subtract, op1=AT.mult)
            # up = (1.5*p < cs); cs_pow = p*(1+up); inv_cs = exact 2^(-y-up)
            ph = const.tile([128, 6], F32)
            DVE.tensor_scalar(out=ph[:], in0=p2[:], scalar1=1.5, scalar2=None,
                              op0=AT.mult)
            upf = const.tile([128, 6], F32)
            DVE.tensor_tensor(out=upf[:], in0=ph[:], in1=cs_a[:], op=AT.is_lt)
            up1 = const.tile([128, 6], F32)
            DVE.tensor_scalar(out=up1[:], in0=upf[:], scalar1=1.0, scalar2=None,
                              op0=AT.add)
            cs_pow = const.tile([128, 6], F32)
            DVE.tensor_tensor(out=cs_pow[:], in0=p2[:], in1=up1[:], op=AT.mult)
            yu = const.tile([128, 6], F32)
            DVE.tensor_tensor(out=yu[:], in0=yf[:], in1=upf[:], op=AT.add)
            inv_cs = const.tile([128, 6], F32)
            ACT.activation(out=inv_cs[:], in_=yu[:], func=AFT.Exp, scale=-LN2)
            DVE.tensor_scalar(out=inv_cs[:], in0=inv_cs[:], scalar1=4096.0,
                              scalar2=RND, op0=AT.mult, op1=AT.add)
            DVE.tensor_scalar(out=inv_cs[:], in0=inv_cs[:], scalar1=RND,
                              scalar2=1.0 / 4096.0, op0=AT.subtract, op1=AT.mult)

            # ---- s1 = rowmax |w1*cs| (ACT computes |w*cs|, DVE max-accumulates;
            #      emitted first so the DVE max chain starts right after AR1) --
            DVE.memset(s1acc[:], 0.0)
            for ct in range(6):
                for hh in range(2):
                    hs = slice(hh * (H // 2), (hh + 1) * (H // 2))
                    at = w1sc.tile([128, H // 2], F32, tag="w1sc")
                    ACT.activation(out=at[:], in_=w1f[ct][:, hs], func=AFT.Abs,
                                   scale=cs_pow[:, ct:ct + 1])
                    DVE.tensor_tensor(out=s1acc[:, hs], in0=s1acc[:, hs],
                                      in1=at[:], op=AT.max)

            # ---- x quant range (on smoothed x) ----
            t6 = const.tile([128, 6], F32)
            t1 = const.tile([128, 1], F32)
            xmax_s = const.tile([128, 1], F32)
            DVE.tensor_tensor(out=t6[:], in0=stat_max[:], in1=inv_cs[:], op=AT.mult)
            DVE.tensor_reduce(out=t1[:], in_=t6[:], axis=AX, op=AT.max)
            GPS.partition_all_reduce(xmax_s[:], t1[:], channels=128, reduce_op=ROP.max)
            DVE.tensor_scalar(out=xmax_s[:], in0=xmax_s[:], scalar1=0.0, scalar2=None,
                              op0=AT.max)
            t6b = const.tile([128, 6], F32)
            t1b = const.tile([128, 1], F32)
            xnm_s = const.tile([128, 1], F32)
            DVE.tensor_tensor(out=t6b[:], in0=stat_nm[:], in1=inv_cs[:], op=AT.mult)
            DVE.tensor_reduce(out=t1b[:], in_=t6b[:], axis=AX, op=AT.max)
            GPS.partition_all_reduce(xnm_s[:], t1b[:], channels=128, reduce_op=ROP.max)
            DVE.tensor_scalar(out=xnm_s[:], in0=xnm_s[:], scalar1=0.0, scalar2=None,
                              op0=AT.max)
            sx = const.tile([128, 1], F32)
            DVE.tensor_tensor(out=sx[:], in0=xmax_s[:], in1=xnm_s[:], op=AT.add)
            div_const(sx, sx, 255.0, eps_clamp=True)
            inv_sx = const.tile([128, 1], F32)
            recip_newton(inv_sx, sx)
            a_x = const.tile([128, 6], F32)
            DVE.tensor_scalar(out=a_x[:], in0=inv_cs[:], scalar1=inv_sx[:, 0:1],
                              scalar2=None, op0=AT.mult)

            # ---- x quant (re-stream; no clip: |round(x/s)| can exceed the
            # asym range only on the handful of global-extreme elements) ----
            for ct in range(6):
                for hh in range(2):
                    off = hh * 832
                    w = min(832, t_pad - off)
                    xs = xqs.tile([128, 832], F32, tag="xs")
                    SYNC.dma_start(out=xs[:, :w],
                                   in_=xT[ct * 128:(ct + 1) * 128, off:off + w])
                    ACT.activation(out=xs[:, :w], in_=xs[:, :w], func=AFT.Copy,
                                   scale=a_x[:, ct:ct + 1], bias=RND)
                    DVE.tensor_scalar(out=xq[ct][:, off:off + w], in0=xs[:, :w],
                                      scalar1=RND, scalar2=None, op0=AT.subtract)

            # ---- s1 partition-reduce; 1/s1 by direct split reciprocal (no DRAM
            #      bounce on the quant-critical path) ----
            GPS.partition_all_reduce(s1b[:], s1acc[:], channels=128,
                                     reduce_op=ROP.max)
            DVE.tensor_scalar(out=s1b[:], in0=s1b[:], scalar1=R127,
                              scalar2=EPS, op0=AT.mult, op1=AT.max)
            # A1 column form via DRAM bounce on the ACT DMA queue (off the
            # quant path; only gelu's epilogue scale needs it)
            ACT.dma_start(out=s1row[:], in_=s1b[0:1, :])
            ACT.dma_start(out=s1col[:],
                          in_=s1row[0:1, :].rearrange("a (k p) -> (a p) k", p=128))
            DVE.tensor_scalar(out=A1[:], in0=s1col[:], scalar1=sx[:, 0:1],
                              scalar2=None, op0=AT.mult)

            # ---- quantize w1 half-by-half so fc1 (which consumes the first
            #      128-column slices first) can start after the first half ----
            for hh in range(2):
                hs = slice(hh * (H // 2), (hh + 1) * (H // 2))
                DVE.reciprocal(out=invs1_bc[:, hs], in_=s1b[:, hs])
                for ct in range(6):
                    wt = w1sc.tile([128, H // 2], F32, tag="w1sc")
                    DVE.scalar_tensor_tensor(out=wt[:], in0=w1f[ct][:, hs],
                                             scalar=cs_pow[:, ct:ct + 1],
                                             in1=invs1_bc[:, hs],
                                             op0=AT.mult, op1=AT.mult)
                    DVE.tensor_scalar(out=w1q[ct][:, hs], in0=wt[:], scalar1=RND,
                                      scalar2=RND, op0=AT.add, op1=AT.subtract)

        # ============ w2 absmax -> s2, 1/s2 (local, no collective) ===============
        w2load_insts = []
        with tc.tile_pool(name="w2s", bufs=3) as w2s, \
             tc.tile_pool(name="s2scr", bufs=1) as s2scr:
            s2acc = s2scr.tile([128, C], F32)
            s2b = s2scr.tile([128, C], F32)
            DVE.memset(s2acc[:], 0.0)
            for kt in range(24):
                wt = w2s.tile([128, C], F32, tag="w2s")
                wl = SYNC.dma_start(out=wt[:], in_=w2T[kt * 128:(kt + 1) * 128, :])
                w2load_insts.append(wl)
                if kt == 0:
                    for pl in w1load_insts:
                        add_dep_helper(wl.ins, pl.ins,
                                       reason="w1 DMA priority over w2")
                ACT.activation(out=wt[:], in_=wt[:], func=AFT.Abs)
                DVE.tensor_tensor(out=s2acc[:], in0=s2acc[:], in1=wt[:],
                                  op=AT.max)
            GPS.partition_all_reduce(s2b[:], s2acc[:], channels=128,
                                     reduce_op=ROP.max)
            SYNC.dma_start(out=s2row[:], in_=s2b[0:1, :])
            SYNC.dma_start(out=s2col[:],
                           in_=s2row[0:1, :].rearrange("a (k p) -> (a p) k", p=128))
            DVE.tensor_scalar(out=s2col[:], in0=s2col[:], scalar1=R127,
                              scalar2=EPS, op0=AT.mult, op1=AT.max)
            DVE.reciprocal(out=i2col[:], in_=s2col[:])
            SYNC.dma_start(out=i2row[0:1, :].rearrange("a (k p) -> (a p) k", p=128),
                           in_=i2col[:])
            SYNC.dma_start(out=invs2_bc[:],
                           in_=i2row[0:1, :].to_broadcast([128, C]))

        # ============ FC1 + GELU; w2 quant emitted after (runs under fc1) ========
        # long-lived pools go on the RIGHT allocator stack so the short-lived
        # prep pools (left stack) can release in LIFO order underneath them
        wqp = ctx.enter_context(tc.tile_pool(name="wqp", bufs=1, side="right"))
        w2qs_pool = tc.tile_pool(name="w2qs", bufs=3)
        w2qs = w2qs_pool.__enter__()
        w2q = [wqp.tile([128, C], BF16, name=f"w2q{i}") for i in range(24)]

        hp = ctx.enter_context(tc.tile_pool(name="hp", bufs=1, side="right"))
        h_res = [hp.tile([128, t_pad], F32, name=f"h{i}") for i in range(NRES)]
        with tc.tile_pool(name="hring", bufs=4) as hring:
            for ht in range(24):
                pst = [psp.tile([128, 512], F32, tag="ps", name=f"ps1_{ht}_{i}")
                       for i in range(len(CH))]
                for ct in range(6):
                    for ci, (off, w) in enumerate(CH):
                        MM(pst[ci][:, :w], lhsT=w1q[ct][:, ht * 128:(ht + 1) * 128],
                           rhs=xq[ct][:, off:off + w], start=(ct == 0),
                           stop=(ct == 5))
                if ht < NRES:
                    for ci, (off, w) in enumerate(CH):
                        ACT.activation(out=h_res[ht][:, off:off + w],
                                       in_=pst[ci][:, :w], func=AFT.Gelu,
                                       scale=A1[:, ht:ht + 1],
                                       bias=b1t[:, ht:ht + 1])
                    DVE.tensor_reduce(out=habs_cols[:, ht:ht + 1],
                                      in_=h_res[ht][:, 0:t_loc], axis=AX,
                                      op=AT.max, apply_absolute_value=True)
                else:
                    idx = ht - NRES
                    for ci, (off, w) in enumerate(CH):
                        hr = hring.tile([128, 512], F32, tag="hr")
                        ACT.activation(out=hr[:, :w], in_=pst[ci][:, :w],
                                       func=AFT.Gelu, scale=A1[:, ht:ht + 1],
                                       bias=b1t[:, ht:ht + 1])
                        wv = max(0, min(w, t_loc - off))
                        DVE.tensor_reduce(
                            out=habs_cols[:, NRES + idx * 4 + ci:NRES + idx * 4 + ci + 1],
                            in_=hr[:, :wv], axis=AX, op=AT.max,
                            apply_absolute_value=True)
                        SYNC.dma_start(out=hd[idx, :, off:off + w], in_=hr[:, :w])

        # ================= h absmax AllReduce -> s_h =================
        # (emitted before the w2 quant loop so AR2 fires the moment fc1 drains)
        hb1 = const.tile([128, 1], F32)
        DVE.tensor_reduce(out=hb1[:], in_=habs_cols[:], axis=AX, op=AT.max)
        habs_r = const.tile([128, 1], F32)
        GPS.partition_all_reduce(habs_r[:], hb1[:], channels=128, reduce_op=ROP.max)
        sc_a = const.tile([1, 8], F32)
        DVE.memset(sc_a[:], 0.0)
        DVE.tensor_copy(out=sc_a[0:1, 0:1], in_=habs_r[0:1, 0:1])
        ACT.dma_start(out=sc_in[:], in_=sc_a[:])
        GPS.collective_compute("AllReduce", AT.max, replica_groups=RG,
                               ins=[sc_in.opt()], outs=[sc_out.opt()])
        s_h = const.tile([128, 1], F32)
        ACT.dma_start(out=s_h[:], in_=sc_out[0:1, 0:1].to_broadcast([128, 1]))
        div_const(s_h, s_h, 127.0, eps_clamp=True)
        inv_sh = const.tile([128, 1], F32)
        recip_newton(inv_sh, s_h)
        # fc2 epilogue per-partition scale: s_h * s2[c] (column form)
        DVE.tensor_scalar(out=es2[:], in0=s2col[:], scalar1=s_h[:, 0:1],
                          scalar2=None, op0=AT.mult)

        # -- quantize w2T (fills the AR2 stall and the fc2 ramp; DVE idle there) --
        for kt in range(24):
            wt = w2qs.tile([128, C], F32, tag="w2s2")
            wl = SYNC.dma_start(out=wt[:], in_=w2T[kt * 128:(kt + 1) * 128, :])
            if kt == 0:
                for pl in w2load_insts:
                    add_dep_helper(wl.ins, pl.ins,
                                   reason="w2 absmax stream priority")
            DVE.tensor_tensor(out=wt[:], in0=wt[:], in1=invs2_bc[:], op=AT.mult)
            DVE.tensor_scalar(out=w2q[kt][:], in0=wt[:], scalar1=RND,
                              scalar2=RND, op0=AT.add, op1=AT.subtract)
        w2qs_pool.__exit__(None, None, None)
        gemm1.close()  # frees xq, w1q (left-stack LIFO: w2qs closed first)

        # ======== FC2 (out in [C, tokens] layout), h quantized just-in-time ======
        outp = ctx.enter_context(tc.tile_pool(name="outp", bufs=1, side="right"))
        out_t = [outp.tile([128, t_pad], F32, name=f"o{i}") for i in range(6)]
        with tc.tile_pool(name="ringA", bufs=4) as ringA, \
             tc.tile_pool(name="tmpp", bufs=4) as tmpp, \
             tc.tile_pool(name="hqB", bufs=6) as hqB:
            for ci, (off, w) in enumerate(CH):
                pst = [psp.tile([128, 512], F32, tag="ps", name=f"ps2_{ci}_{i}")
                       for i in range(6)]
                for ht in range(24):
                    if ht < NRES:
                        src = h_res[ht][:, off:off + w]
                    else:
                        ra = ringA.tile([128, 512], F32, tag="ra")
                        SYNC.dma_start(out=ra[:, :w],
                                       in_=hd[ht - NRES, :, off:off + w])
                        src = ra[:, :w]
                    tq = tmpp.tile([128, 512], F32, tag="tq")
                    ACT.activation(out=tq[:, :w], in_=src, func=AFT.Copy,
                                   scale=inv_sh[:, 0:1], bias=RND)
                    hq = hqB.tile([128, 512], BF16, tag="hq")
                    DVE.tensor_scalar(out=hq[:, :w], in0=tq[:, :w], scalar1=RND,
                                      scalar2=None, op0=AT.subtract)
                    for cs_ in range(6):
                        MM(pst[cs_][:, :w],
                           lhsT=w2q[ht][:, cs_ * 128:(cs_ + 1) * 128],
                           rhs=hq[:, :w], start=(ht == 0), stop=(ht == 23))
                wv = max(0, min(w, t_loc - off))
                for cs_ in range(6):
                    ACT.activation(out=out_t[cs_][:, off:off + w],
                                   in_=pst[cs_][:, :w], func=AFT.Identity,
                                   scale=es2[:, cs_:cs_ + 1],
                                   bias=b2t[:, cs_:cs_ + 1])
                    k = ci * 6 + cs_
                    DVE.tensor_reduce(out=omax_cols[:, k:k + 1],
                                      in_=out_t[cs_][:, off:off + wv], axis=AX,
                                      op=AT.max)
                    DVE.tensor_reduce(out=onm_cols[:, k:k + 1],
                                      in_=out_t[cs_][:, off:off + wv], axis=AX,
                                      op=AT.min, negate=True)

        # ================= out min/max AllReduce -> final quant =================
        om1 = const.tile([128, 1], F32)
        DVE.tensor_reduce(out=om1[:], in_=omax_cols[:], axis=AX, op=AT.max)
        omr = const.tile([128, 1], F32)
        GPS.partition_all_reduce(omr[:], om1[:], channels=128, reduce_op=ROP.max)
        on1 = const.tile([128, 1], F32)
        DVE.tensor_reduce(out=on1[:], in_=onm_cols[:], axis=AX, op=AT.max)
        onr = const.tile([128, 1], F32)
        GPS.partition_all_reduce(onr[:], on1[:], channels=128, reduce_op=ROP.max)
        sc_b = const.tile([1, 8], F32)
        DVE.memset(sc_b[:], 0.0)
        DVE.tensor_copy(out=sc_b[0:1, 0:1], in_=omr[0:1, 0:1])
        DVE.tensor_copy(out=sc_b[0:1, 1:2], in_=onr[0:1, 0:1])
        ACT.dma_start(out=sc_in2[:], in_=sc_b[:])
        GPS.collective_compute("AllReduce", AT.max, replica_groups=RG,
                               ins=[sc_in2.opt()], outs=[sc_out2.opt()])
        omax_a = const.tile([128, 1], F32)
        ACT.dma_start(out=omax_a[:], in_=sc_out2[0:1, 0:1].to_broadcast([128, 1]))
        onm_a = const.tile([128, 1], F32)
        ACT.dma_start(out=onm_a[:], in_=sc_out2[0:1, 1:2].to_broadcast([128, 1]))
        DVE.tensor_scalar(out=omax_a[:], in0=omax_a[:], scalar1=0.0, scalar2=None,
                          op0=AT.max)
        DVE.tensor_scalar(out=onm_a[:], in0=onm_a[:], scalar1=0.0, scalar2=None,
                          op0=AT.max)
        so = const.tile([128, 1], F32)
        DVE.tensor_tensor(out=so[:], in0=omax_a[:], in1=onm_a[:], op=AT.add)
        div_const(so, so, 255.0, eps_clamp=True)
        inv_so = const.tile([128, 1], F32)
        recip_newton(inv_so, so)

        # final fake-quant: so*round(out/so); the asym clip only binds on the
        # handful of global-extreme elements (and pad, which the host drops)
        with tc.tile_pool(name="of", bufs=3) as ofp:
            for cs_ in range(6):
                ot = ofp.tile([128, t_pad], F32, tag="of")
                ACT.activation(out=ot[:], in_=out_t[cs_][:], func=AFT.Copy,
                               scale=inv_so[:, 0:1], bias=RND)
                DVE.tensor_scalar(out=ot[:], in0=ot[:], scalar1=RND,
                                  scalar2=so[:, 0:1], op0=AT.subtract,
                                  op1=AT.mult)
                SYNC.dma_start(out=out_e[cs_ * 128:(cs_ + 1) * 128, :], in_=ot[:])


_NC_CACHE = {}


def _get_nc(n_cores=N_CORES, t_loc=TLOC):
    key = (n_cores, t_loc)
    if key not in _NC_CACHE:
        _NC_CACHE[key] = build(n_cores, t_loc)
    return _NC_CACHE[key]


def _prep_in_maps(x, w1, b1, w2, b2, n_cores=N_CORES):
    t_loc = x.reshape(-1, C).shape[0] // n_cores
    t_pad = ((t_loc + 127) // 128) * 128
    xf = np.ascontiguousarray(x, dtype=np.float32).reshape(-1, C)
    xT_full = xf.T  # [C, TOK]
    w1 = np.ascontiguousarray(w1, dtype=np.float32)
    w2 = np.ascontiguousarray(w2, dtype=np.float32)
    w1T = np.ascontiguousarray(w1.T)
    w2T = np.ascontiguousarray(w2.T)
    b1 = np.ascontiguousarray(b1, dtype=np.float32)
    b2 = np.ascontiguousarray(b2, dtype=np.float32)
    in_maps = []
    for c in range(n_cores):
        sh = np.zeros((C, t_pad), dtype=np.float32)
        sh[:, :t_loc] = xT_full[:, c * t_loc:(c + 1) * t_loc]
        in_maps.append(dict(xT=sh, w1T=w1T, w2T=w2T, b1=b1, b2=b2))
    return in_maps, t_loc


def _install_profile_hook():
    """Provide the antenv.axon_hooks shim this image lacks, so trace=True can
    capture NTFF profiles through libaxon_pjrt."""
    import types
    if "antenv.axon_hooks" in sys.modules:
        return True
    try:
        import antenv
        mod = types.ModuleType("antenv.axon_hooks")
        holder = {}
        mod.set_axon_ntff_profile_hook = lambda h: holder.__setitem__("v", h)
        mod.get_axon_ntff_profile_hook = lambda: holder.get("v")
        sys.modules["antenv.axon_hooks"] = mod
        antenv.axon_hooks = mod
        from trn_agent_boot.trn_boot import _ntff_profile_via_ctypes
        mod.set_axon_ntff_profile_hook(
            _ntff_profile_via_ctypes("/opt/axon/libaxon_pjrt.so"))
        return True
    except Exception as e:  # profiling is best-effort
        print(f"[kernel] profile hook install failed: {e}")
        return False


def kernel(x, w1, b1, w2, b2, trace=False):
    from concourse.bass_utils import run_bass_kernel_spmd

    if trace:
        trace = _install_profile_hook()

    x = np.asarray(x)
    in_maps, t_loc = _prep_in_maps(x, w1, b1, w2, b2)
    nc = _get_nc(N_CORES, t_loc)
    res = run_bass_kernel_spmd(nc, in_maps, core_ids=list(range(N_CORES)),
                               trace=trace)
    out = np.concatenate(
        [np.ascontiguousarray(res.results[c]["out"][:, :t_loc].T)
         for c in range(N_CORES)], axis=0)
    out = out.reshape(x.shape).astype(np.float32)
    kernel.last_results = res
    return out


# revision 31
# speedup vs baseline: 1.0809x; 1.0809x over previous
"""Trainium2 Bass kernel for nn_Mlp_84275848282705 (SmoothQuant-style quantized ViT MLP).

v2: data-parallel over tokens (12608 = 8 x 1576). Changes vs v1 baseline:
- fc2 output computed in transposed [C, tokens] layout: the epilogue
  (s_h*s2[c] scale + b2[c] bias) becomes per-partition Scalar-engine work
  straight out of PSUM, and the output stays in SBUF until the final quant
  (host transposes back). No out DRAM round-trip.
- h (pre-quant gelu output, must stay f32) is kept SBUF-resident for
  NRES of 24 h-tiles; only the rest spill to DRAM. hq (integers, bf16-exact)
  is produced just-in-time per fc2 chunk into a small ring.
- w1 is loaded once (f32-resident through colmax/s1/quant).
- Big-reciprocal fix: 1/s1 computed on a [128,24] column form (DRAM-bounce)
  instead of a 25us DVE iterative divide on [128,3072].
- Symmetric-quant clamps dropped (|round(w/s)| <= 127 by construction).
"""
import sys

sys.path.insert(0, "/opt/trn_rl_repo")

import numpy as np

B, N, C, H = 64, 197, 768, 3072
TOK = B * N             # 12608
N_CORES = 8
TLOC = TOK // N_CORES   # 1576
NRES = 12               # h tiles resident in SBUF (of 24); rest spill to DRAM
RND = 12582912.0        # 1.5*2^23: RNE integer-round magic const (valid for |x| <= 2^22)
EPS = 1e-8
INV_LN2 = 1.4426950408889634
LN2 = 0.6931471805599453
R127 = float(np.float32(1.0) / np.float32(127.0))
R255 = float(np.float32(1.0) / np.float32(255.0))


def _chunks(t_pad, step):
    out, off = [], 0
    while off < t_pad:
        w = min(step, t_pad - off)
        out.append((off, w))
        off += w
    return out


def build(n_cores=N_CORES, t_loc=TLOC):
    import concourse.bacc as bacc
    import concourse.tile as tile
    from concourse import mybir

    F32 = mybir.dt.float32
    t_pad = ((t_loc + 127) // 128) * 128

    nc = bacc.Bacc("TRN2", target_bir_lowering=False, debug=False,
                   enable_asserts=False, num_devices=n_cores)

    io = dict(
        xT=nc.dram_tensor("xT", [C, t_pad], F32, kind="ExternalInput").ap(),
        w1T=nc.dram_tensor("w1T", [C, H], F32, kind="ExternalInput").ap(),
        w2T=nc.dram_tensor("w2T", [H, C], F32, kind="ExternalInput").ap(),
        b1=nc.dram_tensor("b1", [H], F32, kind="ExternalInput").ap(),
        b2=nc.dram_tensor("b2", [C], F32, kind="ExternalInput").ap(),
        # transposed output: [C, t_pad]; host transposes back
        out_e=nc.dram_tensor("out", [C, t_pad], F32, kind="ExternalOutput").ap(),
    )

    with tile.TileContext(nc) as tc:
        _emit(nc, tc, io, n_cores, t_loc, t_pad)
    nc.compile()
    return nc


def _emit(nc, tc, io, n_cores, t_loc, t_pad):
    from contextlib import ExitStack
    from concourse import mybir, bass_isa
    from concourse.tile import add_dep_helper

    F32 = mybir.dt.float32
    BF16 = mybir.dt.bfloat16
    AT = mybir.AluOpType
    AFT = mybir.ActivationFunctionType
    AX = mybir.AxisListType.X
    ROP = bass_isa.ReduceOp
    RG = [list(range(n_cores))]

    xT, w1T, w2T, b1, b2, out_e = (io[k] for k in
                                   ("xT", "w1T", "w2T", "b1", "b2", "out_e"))

    CH = _chunks(t_pad, 512)
    n_spill = 24 - NRES

    DVE, ACT, GPS, SYNC = nc.vector, nc.scalar, nc.gpsimd, nc.sync
    MM = nc.tensor.matmul

    with ExitStack() as ctx:
        const = ctx.enter_context(tc.tile_pool(name="const", bufs=1))
        dram = ctx.enter_context(tc.tile_pool(name="dram", bufs=1, space="DRAM"))
        psp = ctx.enter_context(tc.tile_pool(name="ps", bufs=8, space="PSUM"))

        # DRAM scratch (collectives + layout bounces + h spill)
        st_in = dram.tile([128, 12], F32)
        st_out = dram.tile([128, 12], F32)
        sc_in = dram.tile([1, 8], F32)
        sc_out = dram.tile([1, 8], F32)
        sc_in2 = dram.tile([1, 8], F32)
        sc_out2 = dram.tile([1, 8], F32)
        s1row = dram.tile([1, H], F32)
        s2row = dram.tile([1, C], F32)
        i2row = dram.tile([1, C], F32)
        hd = dram.tile([n_spill, 128, t_pad], F32)

        # small const tiles
        b1t = const.tile([128, 24], F32)
        SYNC.dma_start(out=b1t[:], in_=b1.rearrange("(k p) -> p k", p=128))
        b2t = const.tile([128, 6], F32)
        SYNC.dma_start(out=b2t[:], in_=b2.rearrange("(k p) -> p k", p=128))

        stats12 = const.tile([128, 12], F32)
        stat_max = stats12[:, 0:6]
        stat_nm = stats12[:, 6:12]
        stat_abs = const.tile([128, 6], F32)
        wcol = const.tile([128, 6], F32)
        n_hcols = NRES + 4 * n_spill
        habs_cols = const.tile([128, n_hcols], F32)
        omax_cols = const.tile([128, 24], F32)
        onm_cols = const.tile([128, 24], F32)
        invs2_bc = const.tile([128, C], F32)
        s2col = const.tile([128, 6], F32)
        i2col = const.tile([128, 6], F32)
        es2 = const.tile([128, 6], F32)
        s1col = const.tile([128, 24], F32)
        A1 = const.tile([128, 24], F32)

        # ---- small-tile math helpers (DVE has no divide: reciprocal+Newton) ----
        _mtmp = [0]

        def _tmp(shape):
            t = const.tile(list(shape), F32, name=f"mt{_mtmp[0]}")
            _mtmp[0] += 1
            return t

        def recip_newton(out, b):
            """out = 1/b to ~0.5 ulp (InstReciprocal + one Newton step)."""
            DVE.reciprocal(out=out[:], in_=b[:])
            t = _tmp(b.shape)
            DVE.tensor_tensor(out=t[:], in0=b[:], in1=out[:], op=AT.mult)
            DVE.tensor_scalar(out=t[:], in0=t[:], scalar1=-1.0, scalar2=2.0,
                              op0=AT.mult, op1=AT.add)
            DVE.tensor_tensor(out=out[:], in0=out[:], in1=t[:], op=AT.mult)

        def div_const(out, a, c, eps_clamp=False):
            """out = a / c (python const), correctly rounded via Newton residual."""
            r = float(np.float32(1.0) / np.float32(c))
            q0 = _tmp(a.shape)
            DVE.tensor_scalar(out=q0[:], in0=a[:], scalar1=r, scalar2=None,
                              op0=AT.mult)
            e = _tmp(a.shape)
            DVE.scalar_tensor_tensor(out=e[:], in0=q0[:], scalar=-float(c), in1=a[:],
                                     op0=AT.mult, op1=AT.add)
            DVE.scalar_tensor_tensor(out=out[:], in0=e[:], scalar=r, in1=q0[:],
                                     op0=AT.mult, op1=AT.add)
            if eps_clamp:
                DVE.tensor_scalar(out=out[:], in0=out[:], scalar1=EPS, scalar2=None,
                                  op0=AT.max)

        # ============ Phase A: x stats -> AR1 ====================================
        xload_insts = []
        with tc.tile_pool(name="xs0", bufs=2) as xs0:
            for ct in range(6):
                xt = xs0.tile([128, t_pad], F32, tag="x0")
                xload_insts.append(
                    SYNC.dma_start(out=xt[:], in_=xT[ct * 128:(ct + 1) * 128, :]))
                DVE.tensor_reduce(out=stats12[:, ct:ct + 1], in_=xt[:], axis=AX,
                                  op=AT.max)
                DVE.tensor_reduce(out=stats12[:, 6 + ct:7 + ct], in_=xt[:], axis=AX,
                                  op=AT.min, negate=True)
            # straight [128,12] layout: the AllReduce(max) is elementwise, so
            # no transpose-rearrange is needed on the pack/unpack
            pack_insts = [SYNC.dma_start(out=st_in[:], in_=stats12[:])]
            GPS.collective_compute("AllReduce", AT.max, replica_groups=RG,
                                   ins=[st_in.opt()], outs=[st_out.opt()])
            SYNC.dma_start(out=stats12[:], in_=st_out[:])
            DVE.tensor_tensor(out=stat_abs[:], in0=stat_max[:], in1=stat_nm[:],
                              op=AT.max)

        # ============ Phase B: w1 resident; cs chain; s1; w1 quant; x quant ======
        gemm1 = ExitStack()
        xqp = gemm1.enter_context(tc.tile_pool(name="xqp", bufs=1))
        w1qp = gemm1.enter_context(tc.tile_pool(name="w1qp", bufs=1))
        xq = [xqp.tile([128, t_pad], BF16, name=f"xq{i}") for i in range(6)]
        w1q = [w1qp.tile([128, H], BF16, name=f"w1q{i}") for i in range(6)]
        w1load_insts = []
        with tc.tile_pool(name="w1fp", bufs=1) as w1fp, \
             tc.tile_pool(name="w1sc", bufs=2) as w1sc, \
             tc.tile_pool(name="xqs", bufs=6) as xqs, \
             tc.tile_pool(name="s1scr", bufs=1) as s1scr:
            w1f = [w1fp.tile([128, H], F32, name=f"w1f{i}") for i in range(6)]
            s1acc = s1scr.tile([128, H], F32)
            s1b = s1scr.tile([128, H], F32)
            invs1_bc = s1scr.tile([128, H], F32)

            for ct in range(6):
                wl = SYNC.dma_start(out=w1f[ct][:],
                                    in_=w1T[ct * 128:(ct + 1) * 128, :])
                w1load_insts.append(wl)
                if ct == 0:
                    for xl in xload_insts:
                        add_dep_helper(wl.ins, xl.ins,
                                       reason="x stats DMA priority")
                wr = DVE.tensor_reduce(out=wcol[:, ct:ct + 1], in_=w1f[ct][:],
                                       axis=AX, op=AT.max,
                                       apply_absolute_value=True)
                for pk in pack_insts:
                    add_dep_helper(wr.ins, pk.ins,
                                   reason="AR1 pack before w1 colmax on DVE")

            # ---- channel scale cs = pow2-snap(sqrt(gmax/wmax)) ----
            ratio = const.tile([128, 6], F32)
            rw = const.tile([128, 6], F32)
            DVE.reciprocal(out=rw[:], in_=wcol[:])
            DVE.tensor_tensor(out=ratio[:], in0=stat_abs[:], in1=rw[:], op=AT.mult)
            cs_a = const.tile([128, 6], F32)
            ACT.activation(out=cs_a[:], in_=ratio[:], func=AFT.Sqrt)
            rc = const.tile([128, 6], F32)
            DVE.reciprocal(out=rc[:], in_=cs_a[:])
            newt = const.tile([128, 6], F32)
            DVE.tensor_tensor(out=newt[:], in0=ratio[:], in1=rc[:], op=AT.mult)
            DVE.tensor_tensor(out=cs_a[:], in0=cs_a[:], in1=newt[:], op=AT.add)
            DVE.tensor_scalar(out=cs_a[:], in0=cs_a[:], scalar1=0.5, scalar2=None,
                              op0=AT.mult)
            # y = floor(log2(cs)) = round(ln(cs)*(1/ln2) - 0.5)  (RNE round-trick)
            yf = const.tile([128, 6], F32)
            ACT.activation(out=yf[:], in_=cs_a[:], func=AFT.Ln)
            DVE.tensor_scalar(out=yf[:], in0=yf[:], scalar1=INV_LN2,
                              scalar2=0.5, op0=AT.mult, op1=AT.subtract)
            DVE.tensor_scalar(out=yf[:], in0=yf[:], scalar1=RND, scalar2=RND,
                              op0=AT.add, op1=AT.subtract)
            # p = exact 2^y: exp(y*ln2), snapped to exact value at 2^12 scale
            p2 = const.tile([128, 6], F32)
            ACT.activation(out=p2[:], in_=yf[:], func=AFT.Exp, scale=LN2)
            DVE.tensor_scalar(out=p2[:], in0=p2[:], scalar1=4096.0, scalar2=RND,
                              op0=AT.mult, op1=AT.add)
            DVE.tensor_scalar(out=p2[:], in0=p2[:], scalar1=RND,
                              scalar2=1.0 / 4096.0, op0=AT.subtract, op1=AT.mult)
            # up = (1.5*p < cs); cs_pow = p*(1+up); inv_cs = exact 2^(-y-up)
            ph = const.tile([128, 6], F32)
            DVE.tensor_scalar(out=ph[:], in0=p2[:], scalar1=1.5, scalar2=None,
                              op0=AT.mult)
            upf = const.tile([128, 6], F32)
            DVE.tensor_tensor(out=upf[:], in0=ph[:], in1=cs_a[:], op=AT.is_lt)
            up1 = const.tile([128, 6], F32)
            DVE.tensor_scalar(out=up1[:], in0=upf[:], scalar1=1.0, scalar2=None,
                              op0=AT.add)
            cs_pow = const.tile([128, 6], F32)
            DVE.tensor_tensor(out=cs_pow[:], in0=p2[:], in1=up1[:], op=AT.mult)
            yu = const.tile([128, 6], F32)
            DVE.tensor_tensor(out=yu[:], in0=yf[:], in1=upf[:], op=AT.add)
            inv_cs = const.tile([128, 6], F32)
            ACT.activation(out=inv_cs[:], in_=yu[:], func=AFT.Exp, scale=-LN2)
            DVE.tensor_scalar(out=inv_cs[:], in0=inv_cs[:], scalar1=4096.0,
                              scalar2=RND, op0=AT.mult, op1=AT.add)
            DVE.tensor_scalar(out=inv_cs[:], in0=inv_cs[:], scalar1=RND,
                              scalar2=1.0 / 4096.0, op0=AT.subtract, op1=AT.mult)

            # ---- s1 = rowmax |w1*cs| (ACT computes |w*cs|, DVE max-accumulates;
            #      emitted first so the DVE max chain starts right after AR1) --
            DVE.memset(s1acc[:], 0.0)
            for ct in range(6):
                for hh in range(4):
                    hs = slice(hh * (H // 4), (hh + 1) * (H // 4))
                    at = w1sc.tile([128, H // 4], F32, tag="w1sc")
                    ACT.activation(out=at[:], in_=w1f[ct][:, hs], func=AFT.Abs,
                                   scale=cs_pow[:, ct:ct + 1])
                    DVE.tensor_tensor(out=s1acc[:, hs], in0=s1acc[:, hs],
                                      in1=at[:], op=AT.max)

            # ---- x quant range (on smoothed x) ----
            t6 = const.tile([128, 6], F32)
            t1 = const.tile([128, 1], F32)
            xmax_s = const.tile([128, 1], F32)
            DVE.tensor_tensor(out=t6[:], in0=stat_max[:], in1=inv_cs[:], op=AT.mult)
            DVE.tensor_reduce(out=t1[:], in_=t6[:], axis=AX, op=AT.max)
            GPS.partition_all_reduce(xmax_s[:], t1[:], channels=128, reduce_op=ROP.max)
            DVE.tensor_scalar(out=xmax_s[:], in0=xmax_s[:], scalar1=0.0, scalar2=None,
                              op0=AT.max)
            t6b = const.tile([128, 6], F32)
            t1b = const.tile([128, 1], F32)
            xnm_s = const.tile([128, 1], F32)
            DVE.tensor_tensor(out=t6b[:], in0=stat_nm[:], in1=inv_cs[:], op=AT.mult)
            DVE.tensor_reduce(out=t1b[:], in_=t6b[:], axis=AX, op=AT.max)
            GPS.partition_all_reduce(xnm_s[:], t1b[:], channels=128, reduce_op=ROP.max)
            DVE.tensor_scalar(out=xnm_s[:], in0=xnm_s[:], scalar1=0.0, scalar2=None,
                              op0=AT.max)
            sx = const.tile([128, 1], F32)
            DVE.tensor_tensor(out=sx[:], in0=xmax_s[:], in1=xnm_s[:], op=AT.add)
            DVE.tensor_scalar(out=sx[:], in0=sx[:], scalar1=R255, scalar2=EPS,
                              op0=AT.mult, op1=AT.max)
            inv_sx = const.tile([128, 1], F32)
            DVE.reciprocal(out=inv_sx[:], in_=sx[:])
            a_x = const.tile([128, 6], F32)
            DVE.tensor_scalar(out=a_x[:], in0=inv_cs[:], scalar1=inv_sx[:, 0:1],
                              scalar2=None, op0=AT.mult)

            # ---- x quant (re-stream; no clip: |round(x/s)| can exceed the
            # asym range only on the handful of global-extreme elements) ----
            for ct in range(6):
                for hh in range(2):
                    off = hh * 832
                    w = min(832, t_pad - off)
                    xs = xqs.tile([128, 832], F32, tag="xs")
                    SYNC.dma_start(out=xs[:, :w],
                                   in_=xT[ct * 128:(ct + 1) * 128, off:off + w])
                    ACT.activation(out=xs[:, :w], in_=xs[:, :w], func=AFT.Copy,
                                   scale=a_x[:, ct:ct + 1], bias=RND)
                    DVE.tensor_scalar(out=xq[ct][:, off:off + w], in0=xs[:, :w],
                                      scalar1=RND, scalar2=None, op0=AT.subtract)

            # ---- s1 partition-reduce; 1/s1 by direct split reciprocal (no DRAM
            #      bounce on the quant-critical path) ----
            GPS.partition_all_reduce(s1b[:], s1acc[:], channels=128,
                                     reduce_op=ROP.max)
            DVE.tensor_scalar(out=s1b[:], in0=s1b[:], scalar1=R127,
                              scalar2=EPS, op0=AT.mult, op1=AT.max)
            # A1 column form via DRAM bounce on the ACT DMA queue (off the
            # quant path; only gelu's epilogue scale needs it)
            ACT.dma_start(out=s1row[:], in_=s1b[0:1, :])
            ACT.dma_start(out=s1col[:],
                          in_=s1row[0:1, :].rearrange("a (k p) -> (a p) k", p=128))
            DVE.tensor_scalar(out=A1[:], in0=s1col[:], scalar1=sx[:, 0:1],
                              scalar2=None, op0=AT.mult)

            # ---- quantize w1 quarter-by-quarter so fc1 (which consumes the
            #      low 128-column slices first) starts after the first quarter --
            for qq in range(4):
                hs = slice(qq * (H // 4), (qq + 1) * (H // 4))
                DVE.reciprocal(out=invs1_bc[:, hs], in_=s1b[:, hs])
                for ct in range(6):
                    wt = w1sc.tile([128, H // 4], F32, tag="w1sc")
                    DVE.scalar_tensor_tensor(out=wt[:], in0=w1f[ct][:, hs],
                                             scalar=cs_pow[:, ct:ct + 1],
                                             in1=invs1_bc[:, hs],
                                             op0=AT.mult, op1=AT.mult)
                    DVE.tensor_scalar(out=w1q[ct][:, hs], in0=wt[:], scalar1=RND,
                                      scalar2=RND, op0=AT.add, op1=AT.subtract)

        # ============ w2 absmax -> s2, 1/s2 (local, no collective) ===============
        w2load_insts = []
        with tc.tile_pool(name="w2s", bufs=3) as w2s, \
             tc.tile_pool(name="s2scr", bufs=1) as s2scr:
            s2acc = s2scr.tile([128, C], F32)
            s2b = s2scr.tile([128, C], F32)
            DVE.memset(s2acc[:], 0.0)
            for kt in range(24):
                wt = w2s.tile([128, C], F32, tag="w2s")
                wl = SYNC.dma_start(out=wt[:], in_=w2T[kt * 128:(kt + 1) * 128, :])
                w2load_insts.append(wl)
                if kt == 0:
                    for pl in w1load_insts:
                        add_dep_helper(wl.ins, pl.ins,
                                       reason="w1 DMA priority over w2")
                ACT.activation(out=wt[:], in_=wt[:], func=AFT.Abs)
                DVE.tensor_tensor(out=s2acc[:], in0=s2acc[:], in1=wt[:],
                                  op=AT.max)
            GPS.partition_all_reduce(s2b[:], s2acc[:], channels=128,
                                     reduce_op=ROP.max)
            SYNC.dma_start(out=s2row[:], in_=s2b[0:1, :])
            SYNC.dma_start(out=s2col[:],
                           in_=s2row[0:1, :].rearrange("a (k p) -> (a p) k", p=128))
            DVE.tensor_scalar(out=s2col[:], in0=s2col[:], scalar1=R127,
                              scalar2=EPS, op0=AT.mult, op1=AT.max)
            DVE.reciprocal(out=i2col[:], in_=s2col[:])
            SYNC.dma_start(out=i2row[0:1, :].rearrange("a (k p) -> (a p) k", p=128),
                           in_=i2col[:])
            SYNC.dma_start(out=invs2_bc[:],
                           in_=i2row[0:1, :].to_broadcast([128, C]))

        # ============ FC1 + GELU; w2 quant emitted after (runs under fc1) ========
        # long-lived pools go on the RIGHT allocator stack so the short-lived
        # prep pools (left stack) can release in LIFO order underneath them
        wqp = ctx.enter_context(tc.tile_pool(name="wqp", bufs=1, side="right"))
        w2qs_pool = tc.tile_pool(name="w2qs", bufs=3)
        w2qs = w2qs_pool.__enter__()
        w2q = [wqp.tile([128, C], BF16, name=f"w2q{i}") for i in range(24)]

        hp = ctx.enter_context(tc.tile_pool(name="hp", bufs=1, side="right"))
        h_res = [hp.tile([128, t_pad], F32, name=f"h{i}") for i in range(NRES)]
        with tc.tile_pool(name="hring", bufs=4) as hring:
            for ht in range(24):
                pst = [psp.tile([128, 512], F32, tag="ps", name=f"ps1_{ht}_{i}")
                       for i in range(len(CH))]
                for ct in range(6):
                    for ci, (off, w) in enumerate(CH):
                        MM(pst[ci][:, :w], lhsT=w1q[ct][:, ht * 128:(ht + 1) * 128],
                           rhs=xq[ct][:, off:off + w], start=(ct == 0),
                           stop=(ct == 5))
                if ht < NRES:
                    for ci, (off, w) in enumerate(CH):
                        ACT.activation(out=h_res[ht][:, off:off + w],
                                       in_=pst[ci][:, :w], func=AFT.Gelu,
                                       scale=A1[:, ht:ht + 1],
                                       bias=b1t[:, ht:ht + 1])
                    DVE.tensor_reduce(out=habs_cols[:, ht:ht + 1],
                                      in_=h_res[ht][:, 0:t_loc], axis=AX,
                                      op=AT.max, apply_absolute_value=True)
                else:
                    idx = ht - NRES
                    for ci, (off, w) in enumerate(CH):
                        hr = hring.tile([128, 512], F32, tag="hr")
                        ACT.activation(out=hr[:, :w], in_=pst[ci][:, :w],
                                       func=AFT.Gelu, scale=A1[:, ht:ht + 1],
                                       bias=b1t[:, ht:ht + 1])
                        wv = max(0, min(w, t_loc - off))
                        DVE.tensor_reduce(
                            out=habs_cols[:, NRES + idx * 4 + ci:NRES + idx * 4 + ci + 1],
                            in_=hr[:, :wv], axis=AX, op=AT.max,
                            apply_absolute_value=True)
                        SYNC.dma_start(out=hd[idx, :, off:off + w], in_=hr[:, :w])

        # ================= h absmax AllReduce -> s_h =================
        # (emitted before the w2 quant loop so AR2 fires the moment fc1 drains)
        hb1 = const.tile([128, 1], F32)
        DVE.tensor_reduce(out=hb1[:], in_=habs_cols[:], axis=AX, op=AT.max)
        habs_r = const.tile([128, 1], F32)
        GPS.partition_all_reduce(habs_r[:], hb1[:], channels=128, reduce_op=ROP.max)
        sc_a = const.tile([1, 8], F32)
        DVE.memset(sc_a[:], 0.0)
        DVE.tensor_copy(out=sc_a[0:1, 0:1], in_=habs_r[0:1, 0:1])
        ACT.dma_start(out=sc_in[:], in_=sc_a[:])
        GPS.collective_compute("AllReduce", AT.max, replica_groups=RG,
                               ins=[sc_in.opt()], outs=[sc_out.opt()])
        s_h = const.tile([128, 1], F32)
        ACT.dma_start(out=s_h[:], in_=sc_out[0:1, 0:1].to_broadcast([128, 1]))
        DVE.tensor_scalar(out=s_h[:], in0=s_h[:], scalar1=R127, scalar2=EPS,
                          op0=AT.mult, op1=AT.max)
        inv_sh = const.tile([128, 1], F32)
        DVE.reciprocal(out=inv_sh[:], in_=s_h[:])
        # fc2 epilogue per-partition scale: s_h * s2[c] (column form)
        DVE.tensor_scalar(out=es2[:], in0=s2col[:], scalar1=s_h[:, 0:1],
                          scalar2=None, op0=AT.mult)

        # -- quantize w2T (fills the AR2 stall and the fc2 ramp; DVE idle there) --
        for kt in range(24):
            wt = w2qs.tile([128, C], F32, tag="w2s2")
            wl = SYNC.dma_start(out=wt[:], in_=w2T[kt * 128:(kt + 1) * 128, :])
            if kt == 0:
                for pl in w2load_insts:
                    add_dep_helper(wl.ins, pl.ins,
                                   reason="w2 absmax stream priority")
            DVE.tensor_tensor(out=wt[:], in0=wt[:], in1=invs2_bc[:], op=AT.mult)
            DVE.tensor_scalar(out=w2q[kt][:], in0=wt[:], scalar1=RND,
                              scalar2=RND, op0=AT.add, op1=AT.subtract)
        w2qs_pool.__exit__(None, None, None)
        gemm1.close()  # frees xq, w1q (left-stack LIFO: w2qs closed first)

        # ======== FC2 (out in [C, tokens] layout), h quantized just-in-time ======
        outp = ctx.enter_context(tc.tile_pool(name="outp", bufs=1, side="right"))
        out_t = [outp.tile([128, t_pad], F32, name=f"o{i}") for i in range(6)]
        with tc.tile_pool(name="ringA", bufs=4) as ringA, \
             tc.tile_pool(name="tmpp", bufs=4) as tmpp, \
             tc.tile_pool(name="hqB", bufs=6) as hqB:
            for ci, (off, w) in enumerate(CH):
                pst = [psp.tile([128, 512], F32, tag="ps", name=f"ps2_{ci}_{i}")
                       for i in range(6)]
                for ht in range(24):
                    if ht < NRES:
                        src = h_res[ht][:, off:off + w]
                    else:
                        ra = ringA.tile([128, 512], F32, tag="ra")
                        SYNC.dma_start(out=ra[:, :w],
                                       in_=hd[ht - NRES, :, off:off + w])
                        src = ra[:, :w]
                    tq = tmpp.tile([128, 512], F32, tag="tq")
                    ACT.activation(out=tq[:, :w], in_=src, func=AFT.Copy,
                                   scale=inv_sh[:, 0:1], bias=RND)
                    hq = hqB.tile([128, 512], BF16, tag="hq")
                    DVE.tensor_scalar(out=hq[:, :w], in0=tq[:, :w], scalar1=RND,
                                      scalar2=None, op0=AT.subtract)
                    for cs_ in range(6):
                        MM(pst[cs_][:, :w],
                           lhsT=w2q[ht][:, cs_ * 128:(cs_ + 1) * 128],
                           rhs=hq[:, :w], start=(ht == 0), stop=(ht == 23))
                wv = max(0, min(w, t_loc - off))
                for cs_ in range(6):
                    ACT.activation(out=out_t[cs_][:, off:off + w],
                                   in_=pst[cs_][:, :w], func=AFT.Identity,
                                   scale=es2[:, cs_:cs_ + 1],
                                   bias=b2t[:, cs_:cs_ + 1])
                    k = ci * 6 + cs_
                    DVE.tensor_reduce(out=omax_cols[:, k:k + 1],
                                      in_=out_t[cs_][:, off:off + wv], axis=AX,
                                      op=AT.max)
                    DVE.tensor_reduce(out=onm_cols[:, k:k + 1],
                                      in_=out_t[cs_][:, off:off + wv], axis=AX,
                                      op=AT.min, negate=True)

        # ================= out min/max AllReduce -> final quant =================
        om1 = const.tile([128, 1], F32)
        DVE.tensor_reduce(out=om1[:], in_=omax_cols[:], axis=AX, op=AT.max)
        omr = const.tile([128, 1], F32)
        GPS.partition_all_reduce(omr[:], om1[:], channels=128, reduce_op=ROP.max)
        on1 = const.tile([128, 1], F32)
        DVE.tensor_reduce(out=on1[:], in_=onm_cols[:], axis=AX, op=AT.max)
        onr = const.tile([128, 1], F32)
        GPS.partition_all_reduce(onr[:], on1[:], channels=128, reduce_op=ROP.max)
        sc_b = const.tile([1, 8], F32)
        DVE.memset(sc_b[:], 0.0)
        DVE.tensor_copy(out=sc_b[0:1, 0:1], in_=omr[0:1, 0:1])
        DVE.tensor_copy(out=sc_b[0:1, 1:2], in_=onr[0:1, 0:1])
        ACT.dma_start(out=sc_in2[:], in_=sc_b[:])
        GPS.collective_compute("AllReduce", AT.max, replica_groups=RG,
                               ins=[sc_in2.opt()], outs=[sc_out2.opt()])
        omax_a = const.tile([128, 1], F32)
        ACT.dma_start(out=omax_a[:], in_=sc_out2[0:1, 0:1].to_broadcast([128, 1]))
        onm_a = const.tile([128, 1], F32)
        ACT.dma_start(out=onm_a[:], in_=sc_out2[0:1, 1:2].to_broadcast([128, 1]))
        DVE.tensor_scalar(out=omax_a[:], in0=omax_a[:], scalar1=0.0, scalar2=None,
                          op0=AT.max)
        DVE.tensor_scalar(out=onm_a[:], in0=onm_a[:], scalar1=0.0, scalar2=None,
                          op0=AT.max)
        so = const.tile([128, 1], F32)
        DVE.tensor_tensor(out=so[:], in0=omax_a[:], in1=onm_a[:], op=AT.add)
        DVE.tensor_scalar(out=so[:], in0=so[:], scalar1=R255, scalar2=EPS,
                          op0=AT.mult, op1=AT.max)
        inv_so = const.tile([128, 1], F32)
        DVE.reciprocal(out=inv_so[:], in_=so[:])

        # final fake-quant: so*round(out/so); the asym clip only binds on the
        # handful of global-extreme elements (and pad, which the host drops)
        with tc.tile_pool(name="of", bufs=3) as ofp:
            for cs_ in range(6):
                ot = ofp.tile([128, t_pad], F32, tag="of")
                ACT.activation(out=ot[:], in_=out_t[cs_][:], func=AFT.Copy,
                               scale=inv_so[:, 0:1], bias=RND)
                DVE.tensor_scalar(out=ot[:], in0=ot[:], scalar1=RND,
                                  scalar2=so[:, 0:1], op0=AT.subtract,
                                  op1=AT.mult)
                SYNC.dma_start(out=out_e[cs_ * 128:(cs_ + 1) * 128, :], in_=ot[:])


_NC_CACHE = {}


def _get_nc(n_cores=N_CORES, t_loc=TLOC):
    key = (n_cores, t_loc)
    if key not in _NC_CACHE:
        _NC_CACHE[key] = build(n_cores, t_loc)
    return _NC_CACHE[key]


def _prep_in_maps(x, w1, b1, w2, b2, n_cores=N_CORES):
    t_loc = x.reshape(-1, C).shape[0] // n_cores
    t_pad = ((t_loc + 127) // 128) * 128
    xf = np.ascontiguousarray(x, dtype=np.float32).reshape(-1, C)
    xT_full = xf.T  # [C, TOK]
    w1 = np.ascontiguousarray(w1, dtype=np.float32)
    w2 = np.ascontiguousarray(w2, dtype=np.float32)
    w1T = np.ascontiguousarray(w1.T)
    w2T = np.ascontiguousarray(w2.T)
    b1 = np.ascontiguousarray(b1, dtype=np.float32)
    b2 = np.ascontiguousarray(b2, dtype=np.float32)
    in_maps = []
    for c in range(n_cores):
        sh = np.zeros((C, t_pad), dtype=np.float32)
        sh[:, :t_loc] = xT_full[:, c * t_loc:(c + 1) * t_loc]
        in_maps.append(dict(xT=sh, w1T=w1T, w2T=w2T, b1=b1, b2=b2))
    return in_maps, t_loc


def _install_profile_hook():
    """Provide the antenv.axon_hooks shim this image lacks, so trace=True can
    capture NTFF profiles through libaxon_pjrt."""
    import types
    if "antenv.axon_hooks" in sys.modules:
        return True
    try:
        import antenv
        mod = types.ModuleType("antenv.axon_hooks")
        holder = {}
        mod.set_axon_ntff_profile_hook = lambda h: holder.__setitem__("v", h)
        mod.get_axon_ntff_profile_hook = lambda: holder.get("v")
        sys.modules["antenv.axon_hooks"] = mod
        antenv.axon_hooks = mod
        from trn_agent_boot.trn_boot import _ntff_profile_via_ctypes
        mod.set_axon_ntff_profile_hook(
            _ntff_profile_via_ctypes("/opt/axon/libaxon_pjrt.so"))
        return True
    except Exception as e:  # profiling is best-effort
        print(f"[kernel] profile hook install failed: {e}")
        return False


def kernel(x, w1, b1, w2, b2, trace=False):
    from concourse.bass_utils import run_bass_kernel_spmd

    if trace:
        trace = _install_profile_hook()

    x = np.asarray(x)
    in_maps, t_loc = _prep_in_maps(x, w1, b1, w2, b2)
    nc = _get_nc(N_CORES, t_loc)
    res = run_bass_kernel_spmd(nc, in_maps, core_ids=list(range(N_CORES)),
                               trace=trace)
    out = np.concatenate(
        [np.ascontiguousarray(res.results[c]["out"][:, :t_loc].T)
         for c in range(N_CORES)], axis=0)
    out = out.reshape(x.shape).astype(np.float32)
    kernel.last_results = res
    return out
